# revision 20
# baseline (speedup 1.0000x reference)
"""MoE grouped linear (gmm) kernel for 8 Trainium2 NeuronCores.

Strategy (expert parallel + token load balancing, bf16 compute):
  - Tokens arrive pre-sorted by expert; group_sizes[e] tokens belong to
    expert e. Core e gets weight[e] plus up to C=512 of expert e's tokens
    (the balanced share, T/8). The "all-to-all" routing is host-side
    slicing, since kernel() sees the full inputs.
  - Excess tokens of heavy experts (g_e > C) are split into guest items of
    (<=128 tokens x one half of the output dim) and scattered one per core,
    so every core does the same 512+64-column-equivalent of PE work instead
    of padding everyone to max(g_e). Guest outputs are final values for
    their (token, out) rectangle - no cross-core reduction.
  - X and W are converted to bf16 (round-to-nearest) host-side, halving
    HBM->SBUF traffic; the PE computes bf16 x bf16 -> fp32 PSUM. Per-core
    DMA (~15.7 MB, ~44 us) then hides fully under PE time (~61 us).
  - Group 0 (o-blocks 0..3) runs k-major; its weights are packed k-major
    host-side and stream as 262 KB k-pair slices on the scalar HW-DGE ring
    while X streams on the sync ring, so the matmul stream is dense from
    the moment the HAM warmup ends. Later weights ride 1 MB pair DMAs.
  - The per-partition bias is fused into the PSUM evacuation instruction.
Host then scatters per-core main/guest outputs back to [T, Out] fp32.
"""

import numpy as np
import ml_dtypes

import concourse.bass as bass
from concourse import bacc
import concourse.mybir as mybir
import concourse.tile as tile
from concourse.bass_utils import run_bass_kernel_spmd

N_CORES = 8
P = 128
GT = 128   # guest item token width
GRP0 = 6   # o-blocks in the k-major group 0

_BUILD_CACHE: dict = {}


def _build_program(c_main: int, n_in: int, n_out: int, s_guest: int):
    kb = n_in // P   # contraction blocks
    ob = n_out // P  # output-row blocks
    oh = ob // 2     # guest o-blocks (half the output dim)
    f32 = mybir.dt.float32
    bf16 = mybir.dt.bfloat16

    nc = bacc.Bacc(
        "TRN2", target_bir_lowering=False, debug=False, num_devices=N_CORES
    )
    # X^T pre-tiled host-side: [P(k within block), kb, C] so each quarter
    # DMA is fully contiguous per partition.
    xm = nc.dram_tensor("xm", [P, kb, c_main], bf16, kind="ExternalInput")
    # Group-0 weights (o-blocks 0..GRP-1) packed k-major: [kb, P, GRP, P].
    # GRP=6 o-blocks per k-slab drops the k-major stream's DMA demand to
    # ~250 KB/us, matching what the SDMA path delivers while it ramps.
    GRP = GRP0
    wk0 = nc.dram_tensor("wk0", [kb, P, GRP, P], bf16, kind="ExternalInput")
    # Remaining o-slabs: [ob-GRP, P(k within block), kb, P(o)].
    wm = nc.dram_tensor("wm", [ob - GRP, P, kb, P], bf16, kind="ExternalInput")
    biasm = nc.dram_tensor("biasm", [P, ob], f32, kind="ExternalInput")
    ym = nc.dram_tensor("ym", [n_out, c_main], f32, kind="ExternalOutput")
    if s_guest:
        xg = nc.dram_tensor("xg", [s_guest, P, kb, GT], bf16, kind="ExternalInput")
        wg = nc.dram_tensor(
            "wg", [s_guest, oh, P, kb, P], bf16, kind="ExternalInput"
        )
        biasg = nc.dram_tensor("biasg", [s_guest, P, oh], f32, kind="ExternalInput")
        yg = nc.dram_tensor(
            "yg", [s_guest, oh * P, GT], f32, kind="ExternalOutput"
        )

    XQ = 2  # k-slabs per X tile (262 KB units match the wk pair slices)

    with tile.TileContext(nc) as tc:
        with (
            tc.tile_pool(name="const", bufs=1) as constp,
            tc.tile_pool(name="xtsb", bufs=1) as xtp,
            tc.tile_pool(name="wksb", bufs=kb // 2) as wkp,
            tc.tile_pool(name="wsb", bufs=(ob - GRP) // 2) as wp,
            tc.tile_pool(name="wgsb", bufs=4) as wgp,
            tc.tile_pool(name="outsb", bufs=2 * GRP) as outp,
            tc.tile_pool(name="goutsb", bufs=4) as goutp,
            tc.tile_pool(name="psum", bufs=1, space="PSUM") as psump,
        ):
            # All compute-critical loads ride the sync HW-DGE ring (the
            # scalar ring ramps ~2 us later at kernel start), interleaved
            # in exactly the k-major consumption order. The scalar ring
            # carries only the small consts, guest X, and output stores.
            wks = [None] * (kb // 2)
            xqs = [None] * (kb // XQ)

            def load_wk(q):
                t = wkp.tile([P, 2, GRP, P], bf16, tag="wk", name=f"wk{q}")
                nc.sync.dma_start(
                    t[:], wk0[2 * q : 2 * q + 2].rearrange("k p o j -> p k o j")
                )
                wks[q] = t

            def load_xq(i):
                t = xtp.tile([P, XQ, c_main], bf16, tag=f"xq{i}", name=f"xq{i}")
                nc.sync.dma_start(t[:], xm[:, i * XQ : (i + 1) * XQ, :])
                xqs[i] = t

            for q in range(kb // 2):
                load_wk(q)
                if q < kb // XQ:
                    load_xq(q)
            bias_sb = constp.tile([P, ob], f32)
            nc.scalar.dma_start(bias_sb[:], biasm[:])
            if s_guest:
                biasg_sb = constp.tile([P, s_guest * oh], f32)
                nc.scalar.dma_start(
                    biasg_sb[:], biasg.rearrange("s p o -> p (s o)")
                )
                xg_sb = []
                for s in range(s_guest):
                    t = xtp.tile([P, kb, GT], bf16, tag=f"xg{s}", name=f"xg{s}")
                    nc.scalar.dma_start(t[:], xg[s])
                    xg_sb.append(t)

            # HAM warmup: dummy matmuls with no data deps run while the
            # prologue DMAs stream, so the PE clock is at 2.4 GHz (and the
            # activity window warm) when the first real matmul issues.
            warm = constp.tile([P, 512], bf16)
            nc.vector.memset(warm[:], 0)
            ps_warm = psump.tile([P, 512], f32, tag="ps0", name="warmps")
            for i in range(8):
                nc.tensor.matmul(
                    ps_warm[:],
                    warm[:, :P],
                    warm[:],
                    start=(i == 0),
                    stop=(i == 7),
                )

            def evac(ps, bias_col, dst, n_cols, engine, name):
                """PSUM -> SBUF with fused per-o bias, then store. Guest
                stores ride the sync ring (idle by then), so the final
                store triggers overlap the evac engines instead of
                serializing behind them on scalar."""
                pool = outp if n_cols == c_main else goutp
                ot = pool.tile([P, n_cols], f32, tag=f"ot{n_cols}", name=name)
                if engine == 0:
                    nc.scalar.activation(
                        ot[:],
                        ps,
                        mybir.ActivationFunctionType.Identity,
                        bias=bias_col,
                    )
                else:
                    nc.vector.tensor_scalar_add(ot[:], ps, bias_col)
                if n_cols == c_main:
                    ring = nc.scalar
                else:
                    # guest stores: pick the ring whose engine is NOT doing
                    # this evac, so the final store triggers run in parallel
                    ring = nc.sync if engine == 0 else nc.scalar
                ring.dma_start(dst, ot[:])

            # All accumulators come from one strict round-robin over GRP
            # full PSUM banks: every bank's next user is GRP o-blocks
            # behind its evacuation, so no phase stalls on a bank. Guests
            # write only the first GT columns of a full-bank tile.
            ps_seq = [0]

            def next_ps(name):
                t = psump.tile(
                    [P, c_main], f32, tag=f"ps{ps_seq[0] % GRP}", name=name
                )
                ps_seq[0] += 1
                return t

            # Group 0 (o-blocks 0..GRP-1) runs k-major so every arriving
            # X-slab immediately feeds GRP o-columns of PE work.
            g0 = list(range(GRP))
            pss0 = {oi: next_ps(f"ps{oi}") for oi in g0}
            for k in range(kb):
                xvk = xqs[k // XQ][:, k % XQ, :]
                wvk = wks[k // 2]
                for oi in g0:
                    nc.tensor.matmul(
                        pss0[oi][:],
                        wvk[:, k % 2, oi, :],
                        xvk,
                        start=(k == 0),
                        stop=(k == kb - 1),
                    )
            for oi in g0:
                evac(
                    pss0[oi][:], bias_sb[:, oi : oi + 1],
                    ym[oi * P : (oi + 1) * P, :], c_main, oi % 2, f"ot{oi}",
                )

            # Remaining o-blocks run one at a time off 1 MB pair DMAs;
            # evacuations pipeline under the next bank's MMs.
            for pi in range((ob - GRP) // 2):
                wpair = wp.tile([P, 2, kb, P], bf16, tag="wo", name=f"wp{pi}")
                nc.sync.dma_start(
                    wpair[:],
                    wm[2 * pi : 2 * pi + 2].rearrange("o p k j -> p o k j"),
                )
                for j in (0, 1):
                    o = GRP + 2 * pi + j
                    ps = next_ps(f"ps{o}")
                    for k in range(kb):
                        nc.tensor.matmul(
                            ps[:],
                            wpair[:, j, k, :],
                            xqs[k // XQ][:, k % XQ, :],
                            start=(k == 0),
                            stop=(k == kb - 1),
                        )
                    evac(
                        ps[:], bias_sb[:, o : o + 1],
                        ym[o * P : (o + 1) * P, :], c_main, o % 2, f"ot{o}",
                    )

            # Guest pass: s_guest items of (GT tokens x oh o-blocks), final
            # outputs for their rectangle (no cross-core reduction).
            if s_guest:
                for s in range(s_guest):
                    for gp in range(oh // 2):
                        wgpair = wgp.tile(
                            [P, 2, kb, P], bf16, tag="wg", name=f"wg{s}_{gp}"
                        )
                        nc.sync.dma_start(
                            wgpair[:],
                            wg[s, 2 * gp : 2 * gp + 2].rearrange(
                                "o p k j -> p o k j"
                            ),
                        )
                        for j in (0, 1):
                            o = 2 * gp + j
                            ps = next_ps(f"psg{s}_{o}")
                            for k in range(kb):
                                nc.tensor.matmul(
                                    ps[:, :GT],
                                    wgpair[:, j, k, :],
                                    xg_sb[s][:, k, :],
                                    start=(k == 0),
                                    stop=(k == kb - 1),
                                )
                            evac(
                                ps[:, :GT],
                                biasg_sb[:, s * oh + o : s * oh + o + 1],
                                yg[s, o * P : (o + 1) * P, :], GT, o % 2,
                                f"go{s}_{o}",
                            )
    nc.finalize()
    return nc


def _tile_x(x_cols: np.ndarray, n_in: int, width: int) -> np.ndarray:
    """[n, n_in] fp32 token rows -> [P, kb, width] bf16 X^T tiling."""
    kb = n_in // P
    xt = np.zeros((n_in, width), np.float32)
    xt[:, : x_cols.shape[0]] = x_cols.T
    return np.ascontiguousarray(
        xt.reshape(kb, P, width).transpose(1, 0, 2)
    ).astype(ml_dtypes.bfloat16)


def _tile_w(w: np.ndarray) -> np.ndarray:
    """[n_in, n_o] fp32 -> [n_o//P, P, n_in//P, P] bf16 o-slab tiling."""
    n_in, n_o = w.shape
    return np.ascontiguousarray(
        w.reshape(n_in // P, P, n_o // P, P).transpose(2, 1, 0, 3)
    ).astype(ml_dtypes.bfloat16)


def _prepare(inputs, weight, bias, group_sizes):
    """Build (or reuse) the program and the per-core input maps."""
    inputs = np.ascontiguousarray(np.asarray(inputs, dtype=np.float32))
    weight = np.ascontiguousarray(np.asarray(weight, dtype=np.float32))
    bias = np.ascontiguousarray(np.asarray(bias, dtype=np.float32))
    g = np.asarray(group_sizes).astype(np.int64)

    t_tokens, n_in = inputs.shape
    n_exp, _, n_out = weight.shape
    assert n_exp == N_CORES, f"expected {N_CORES} experts, got {n_exp}"
    offs = np.concatenate([[0], np.cumsum(g)])
    assert offs[-1] == t_tokens, "group_sizes must sum to token count"

    kb, ob, oh = n_in // P, n_out // P, n_out // P // 2
    GRP = GRP0
    c_main = max(P, -(-t_tokens // N_CORES // P) * P)  # balanced share

    # Guest items: (expert, token-block, out-half) for tokens beyond c_main.
    items = []
    for e in range(n_exp):
        nblk = -(-max(0, int(g[e]) - c_main) // GT)
        for b in range(nblk):
            for h in range(2):
                items.append((e, b, h))
    s_guest = -(-len(items) // N_CORES) if items else 0

    key = (c_main, n_in, n_out, s_guest)
    if key not in _BUILD_CACHE:
        _BUILD_CACHE[key] = _build_program(c_main, n_in, n_out, s_guest)
    nc = _BUILD_CACHE[key]

    biasm_host = np.ascontiguousarray(bias.reshape(ob, P).T)  # [P, ob]

    # slot-major assignment: item i -> (core i % 8, slot i // 8)
    plan = [[None] * s_guest for _ in range(n_exp)]
    for i, it in enumerate(items):
        plan[i % N_CORES][i // N_CORES] = it

    in_maps = []
    for e in range(n_exp):
        nm = min(int(g[e]), c_main)
        wt = weight[e].reshape(kb, P, ob, P)
        m = {
            "xm": _tile_x(inputs[offs[e] : offs[e] + nm], n_in, c_main),
            "wk0": np.ascontiguousarray(wt[:, :, :GRP, :]).astype(
                ml_dtypes.bfloat16
            ),
            "wm": np.ascontiguousarray(
                wt[:, :, GRP:, :].transpose(2, 1, 0, 3)
            ).astype(ml_dtypes.bfloat16),
            "biasm": biasm_host,
        }
        if s_guest:
            xg_l, wg_l, bg_l = [], [], []
            for it in plan[e]:
                if it is None:
                    xg_l.append(np.zeros((P, kb, GT), ml_dtypes.bfloat16))
                    wg_l.append(np.zeros((oh, P, kb, P), ml_dtypes.bfloat16))
                    bg_l.append(np.zeros((P, oh), np.float32))
                else:
                    ge, b, h = it
                    t0 = offs[ge] + c_main + b * GT
                    n = min(GT, offs[ge + 1] - t0)
                    xg_l.append(_tile_x(inputs[t0 : t0 + n], n_in, GT))
                    wg_l.append(
                        _tile_w(weight[ge][:, h * oh * P : (h + 1) * oh * P])
                    )
                    bg_l.append(
                        np.ascontiguousarray(
                            bias.reshape(ob, P)[h * oh : (h + 1) * oh].T
                        )
                    )
            m["xg"] = np.stack(xg_l)
            m["wg"] = np.stack(wg_l)
            m["biasg"] = np.stack(bg_l)
        in_maps.append(m)
    meta = (g, offs, plan, c_main, s_guest)
    return nc, in_maps, meta, None, (t_tokens, n_out)


def kernel(inputs, weight, bias, group_sizes):
    nc, in_maps, meta, _, (t_tokens, n_out) = _prepare(
        inputs, weight, bias, group_sizes
    )
    g, offs, plan, c_main, s_guest = meta
    oh = n_out // P // 2
    res = run_bass_kernel_spmd(nc, in_maps, core_ids=list(range(N_CORES)))

    out = np.empty((t_tokens, n_out), np.float32)
    for e in range(N_CORES):
        nm = min(int(g[e]), c_main)
        if nm:
            out[offs[e] : offs[e] + nm] = res.results[e]["ym"][:, :nm].T
        for s in range(s_guest):
            it = plan[e][s]
            if it is None:
                continue
            ge, b, h = it
            t0 = offs[ge] + c_main + b * GT
            n = min(GT, int(offs[ge + 1] - t0))
            out[t0 : t0 + n, h * oh * P : (h + 1) * oh * P] = res.results[e][
                "yg"
            ][s][:, :n].T
    return out


# revision 21
# speedup vs baseline: 1.0227x; 1.0227x over previous
"""MoE grouped linear (gmm) kernel for 8 Trainium2 NeuronCores.

Strategy (expert parallel + token load balancing, bf16 compute):
  - Tokens arrive pre-sorted by expert; group_sizes[e] tokens belong to
    expert e. Core e gets weight[e] plus up to C=512 of expert e's tokens
    (the balanced share, T/8). The "all-to-all" routing is host-side
    slicing, since kernel() sees the full inputs.
  - Excess tokens of heavy experts (g_e > C) are split into guest items of
    (<=128 tokens x one half of the output dim) and scattered one per core,
    so every core does the same 512+64-column-equivalent of PE work instead
    of padding everyone to max(g_e). Guest outputs are final values for
    their (token, out) rectangle - no cross-core reduction.
  - X and W are converted to bf16 (round-to-nearest) host-side, halving
    HBM->SBUF traffic; the PE computes bf16 x bf16 -> fp32 PSUM. Per-core
    DMA (~15.7 MB, ~44 us) then hides fully under PE time (~61 us).
  - Group 0 (o-blocks 0..3) runs k-major; its weights are packed k-major
    host-side and stream as 262 KB k-pair slices on the scalar HW-DGE ring
    while X streams on the sync ring, so the matmul stream is dense from
    the moment the HAM warmup ends. Later weights ride 1 MB pair DMAs.
  - The per-partition bias is fused into the PSUM evacuation instruction.
Host then scatters per-core main/guest outputs back to [T, Out] fp32.
"""

import numpy as np
import ml_dtypes

import concourse.bass as bass
from concourse import bacc
import concourse.mybir as mybir
import concourse.tile as tile
from concourse.bass_utils import run_bass_kernel_spmd

N_CORES = 8
P = 128
GT = 128  # guest item token width

_BUILD_CACHE: dict = {}


def _build_program(c_main: int, n_in: int, n_out: int, s_guest: int):
    kb = n_in // P   # contraction blocks
    ob = n_out // P  # output-row blocks
    oh = ob // 2     # guest o-blocks (half the output dim)
    f32 = mybir.dt.float32
    bf16 = mybir.dt.bfloat16

    nc = bacc.Bacc(
        "TRN2", target_bir_lowering=False, debug=False, num_devices=N_CORES
    )
    # X^T pre-tiled host-side: [P(k within block), kb, C] so each quarter
    # DMA is fully contiguous per partition.
    xm = nc.dram_tensor("xm", [P, kb, c_main], bf16, kind="ExternalInput")
    # Group-0 weights (o-blocks 0..GRP-1) packed k-major: [kb, P, GRP, P].
    GRP = 4
    wk0 = nc.dram_tensor("wk0", [kb, P, GRP, P], bf16, kind="ExternalInput")
    # Remaining o-slabs: [ob-GRP, P(k within block), kb, P(o)].
    wm = nc.dram_tensor("wm", [ob - GRP, P, kb, P], bf16, kind="ExternalInput")
    biasm = nc.dram_tensor("biasm", [P, ob], f32, kind="ExternalInput")
    ym = nc.dram_tensor("ym", [n_out, c_main], f32, kind="ExternalOutput")
    if s_guest:
        xg = nc.dram_tensor("xg", [s_guest, P, kb, GT], bf16, kind="ExternalInput")
        wg = nc.dram_tensor(
            "wg", [s_guest, oh, P, kb, P], bf16, kind="ExternalInput"
        )
        biasg = nc.dram_tensor("biasg", [s_guest, P, oh], f32, kind="ExternalInput")
        yg = nc.dram_tensor(
            "yg", [s_guest, oh * P, GT], f32, kind="ExternalOutput"
        )

    XQ = 2  # k-slabs per X tile (262 KB units match the wk pair slices)

    with tile.TileContext(nc) as tc:
        with (
            tc.tile_pool(name="const", bufs=1) as constp,
            tc.tile_pool(name="xtsb", bufs=1) as xtp,
            tc.tile_pool(name="wksb", bufs=kb // 2) as wkp,
            tc.tile_pool(name="wsb", bufs=(ob - GRP) // 2) as wp,
            tc.tile_pool(name="wgsb", bufs=4) as wgp,
            tc.tile_pool(name="outsb", bufs=2 * GRP) as outp,
            tc.tile_pool(name="goutsb", bufs=4) as goutp,
            tc.tile_pool(name="psum", bufs=1, space="PSUM") as psump,
        ):
            # All compute-critical loads ride the sync HW-DGE ring (the
            # scalar ring ramps ~2 us later at kernel start), interleaved
            # in exactly the k-major consumption order. The scalar ring
            # carries only the small consts, guest X, and output stores.
            wks = [None] * (kb // 2)
            xqs = [None] * (kb // XQ)

            def load_wk(q):
                t = wkp.tile([P, 2, GRP, P], bf16, tag="wk", name=f"wk{q}")
                nc.sync.dma_start(
                    t[:], wk0[2 * q : 2 * q + 2].rearrange("k p o j -> p k o j")
                )
                wks[q] = t

            def load_xq(i):
                t = xtp.tile([P, XQ, c_main], bf16, tag=f"xq{i}", name=f"xq{i}")
                nc.sync.dma_start(t[:], xm[:, i * XQ : (i + 1) * XQ, :])
                xqs[i] = t

            for q in range(kb // 2):
                load_wk(q)
                if q < kb // XQ:
                    load_xq(q)
            bias_sb = constp.tile([P, ob], f32)
            nc.scalar.dma_start(bias_sb[:], biasm[:])
            if s_guest:
                biasg_sb = constp.tile([P, s_guest * oh], f32)
                nc.scalar.dma_start(
                    biasg_sb[:], biasg.rearrange("s p o -> p (s o)")
                )
                xg_sb = []
                for s in range(s_guest):
                    t = xtp.tile([P, kb, GT], bf16, tag=f"xg{s}", name=f"xg{s}")
                    nc.scalar.dma_start(t[:], xg[s])
                    xg_sb.append(t)

            # HAM warmup: dummy matmuls with no data deps run while the
            # prologue DMAs stream, so the PE clock is at 2.4 GHz (and the
            # activity window warm) when the first real matmul issues.
            warm = constp.tile([P, 512], bf16)
            nc.vector.memset(warm[:], 0)
            ps_warm = psump.tile([P, 512], f32, tag="ps0", name="warmps")
            # 10 warmup MMs (~4.3 us cold): ends right when the first real
            # operands land, with >3.4 us of contiguous PE busy, so the HAM
            # activity window is saturated and the real stream starts at
            # the full 2.4 GHz clock (8 MMs left a 1.1 us idle gap that
            # diluted the window; ~16 real MMs then ran at 1.2 GHz).
            for i in range(10):
                nc.tensor.matmul(
                    ps_warm[:],
                    warm[:, :P],
                    warm[:],
                    start=(i == 0),
                    stop=(i == 9),
                )

            def evac(ps, bias_col, dst, n_cols, engine, name):
                """PSUM -> SBUF with fused per-o bias, then store. Guest
                stores ride the sync ring (idle by then), so the final
                store triggers overlap the evac engines instead of
                serializing behind them on scalar."""
                pool = outp if n_cols == c_main else goutp
                ot = pool.tile([P, n_cols], f32, tag=f"ot{n_cols}", name=name)
                if engine == 0:
                    nc.scalar.activation(
                        ot[:],
                        ps[:],
                        mybir.ActivationFunctionType.Identity,
                        bias=bias_col,
                    )
                else:
                    nc.vector.tensor_scalar_add(ot[:], ps[:], bias_col)
                if n_cols == c_main:
                    ring = nc.scalar
                else:
                    # guest stores: pick the ring whose engine is NOT doing
                    # this evac, so the final store triggers run in parallel
                    ring = nc.sync if engine == 0 else nc.scalar
                ring.dma_start(dst, ot[:])

            # Group 0 (o-blocks 0..GRP-1) runs k-major so every arriving
            # X-slab immediately feeds GRP o-columns of PE work.
            g0 = list(range(GRP))
            pss0 = {
                oi: psump.tile([P, c_main], f32, tag=f"ps{oi}", name=f"ps{oi}")
                for oi in g0
            }
            for k in range(kb):
                xvk = xqs[k // XQ][:, k % XQ, :]
                wvk = wks[k // 2]
                for oi in g0:
                    nc.tensor.matmul(
                        pss0[oi][:],
                        wvk[:, k % 2, oi, :],
                        xvk,
                        start=(k == 0),
                        stop=(k == kb - 1),
                    )
            for oi in g0:
                evac(
                    pss0[oi], bias_sb[:, oi : oi + 1],
                    ym[oi * P : (oi + 1) * P, :], c_main, oi % 2, f"ot{oi}",
                )

            # Remaining o-blocks run one at a time off 1 MB pair DMAs:
            # per-bank k-runs rotate through the PSUM banks (released by
            # group 0 in the same order), and evacuations pipeline under
            # the next bank's MMs.
            for pi in range((ob - GRP) // 2):
                wpair = wp.tile([P, 2, kb, P], bf16, tag="wo", name=f"wp{pi}")
                nc.sync.dma_start(
                    wpair[:],
                    wm[2 * pi : 2 * pi + 2].rearrange("o p k j -> p o k j"),
                )
                for j in (0, 1):
                    o = GRP + 2 * pi + j
                    # 5-bank rotation starting on the fresh ps4 bank, so o4
                    # does not wait for group 0's first evacuation.
                    ps = psump.tile(
                        [P, c_main], f32,
                        tag=f"ps{(o - GRP + 4) % 5}", name=f"ps{o}",
                    )
                    for k in range(kb):
                        nc.tensor.matmul(
                            ps[:],
                            wpair[:, j, k, :],
                            xqs[k // XQ][:, k % XQ, :],
                            start=(k == 0),
                            stop=(k == kb - 1),
                        )
                    evac(
                        ps, bias_sb[:, o : o + 1],
                        ym[o * P : (o + 1) * P, :], c_main, o % 2, f"ot{o}",
                    )

            # Guest pass: s_guest items of (GT tokens x oh o-blocks), final
            # outputs for their rectangle (no cross-core reduction).
            if s_guest:
                for s in range(s_guest):
                    for gp in range(oh // 2):
                        wgpair = wgp.tile(
                            [P, 2, kb, P], bf16, tag="wg", name=f"wg{s}_{gp}"
                        )
                        nc.sync.dma_start(
                            wgpair[:],
                            wg[s, 2 * gp : 2 * gp + 2].rearrange(
                                "o p k j -> p o k j"
                            ),
                        )
                        for j in (0, 1):
                            o = 2 * gp + j
                            ps = psump.tile(
                                [P, GT], f32, tag=f"psg{o % 3}",
                                name=f"psg{s}_{o}",
                            )
                            for k in range(kb):
                                nc.tensor.matmul(
                                    ps[:],
                                    wgpair[:, j, k, :],
                                    xg_sb[s][:, k, :],
                                    start=(k == 0),
                                    stop=(k == kb - 1),
                                )
                            evac(
                                ps,
                                biasg_sb[:, s * oh + o : s * oh + o + 1],
                                yg[s, o * P : (o + 1) * P, :], GT, o % 2,
                                f"go{s}_{o}",
                            )
    nc.finalize()
    return nc


def _tile_x(x_cols: np.ndarray, n_in: int, width: int) -> np.ndarray:
    """[n, n_in] fp32 token rows -> [P, kb, width] bf16 X^T tiling."""
    kb = n_in // P
    xt = np.zeros((n_in, width), np.float32)
    xt[:, : x_cols.shape[0]] = x_cols.T
    return np.ascontiguousarray(
        xt.reshape(kb, P, width).transpose(1, 0, 2)
    ).astype(ml_dtypes.bfloat16)


def _tile_w(w: np.ndarray) -> np.ndarray:
    """[n_in, n_o] fp32 -> [n_o//P, P, n_in//P, P] bf16 o-slab tiling."""
    n_in, n_o = w.shape
    return np.ascontiguousarray(
        w.reshape(n_in // P, P, n_o // P, P).transpose(2, 1, 0, 3)
    ).astype(ml_dtypes.bfloat16)


def _prepare(inputs, weight, bias, group_sizes):
    """Build (or reuse) the program and the per-core input maps."""
    inputs = np.ascontiguousarray(np.asarray(inputs, dtype=np.float32))
    weight = np.ascontiguousarray(np.asarray(weight, dtype=np.float32))
    bias = np.ascontiguousarray(np.asarray(bias, dtype=np.float32))
    g = np.asarray(group_sizes).astype(np.int64)

    t_tokens, n_in = inputs.shape
    n_exp, _, n_out = weight.shape
    assert n_exp == N_CORES, f"expected {N_CORES} experts, got {n_exp}"
    offs = np.concatenate([[0], np.cumsum(g)])
    assert offs[-1] == t_tokens, "group_sizes must sum to token count"

    kb, ob, oh = n_in // P, n_out // P, n_out // P // 2
    GRP = 4
    c_main = max(P, -(-t_tokens // N_CORES // P) * P)  # balanced share

    # Guest items: (expert, token-block, out-half) for tokens beyond c_main.
    items = []
    for e in range(n_exp):
        nblk = -(-max(0, int(g[e]) - c_main) // GT)
        for b in range(nblk):
            for h in range(2):
                items.append((e, b, h))
    s_guest = -(-len(items) // N_CORES) if items else 0

    key = (c_main, n_in, n_out, s_guest)
    if key not in _BUILD_CACHE:
        _BUILD_CACHE[key] = _build_program(c_main, n_in, n_out, s_guest)
    nc = _BUILD_CACHE[key]

    biasm_host = np.ascontiguousarray(bias.reshape(ob, P).T)  # [P, ob]

    # slot-major assignment: item i -> (core i % 8, slot i // 8)
    plan = [[None] * s_guest for _ in range(n_exp)]
    for i, it in enumerate(items):
        plan[i % N_CORES][i // N_CORES] = it

    in_maps = []
    for e in range(n_exp):
        nm = min(int(g[e]), c_main)
        wt = weight[e].reshape(kb, P, ob, P)
        m = {
            "xm": _tile_x(inputs[offs[e] : offs[e] + nm], n_in, c_main),
            "wk0": np.ascontiguousarray(wt[:, :, :GRP, :]).astype(
                ml_dtypes.bfloat16
            ),
            "wm": np.ascontiguousarray(
                wt[:, :, GRP:, :].transpose(2, 1, 0, 3)
            ).astype(ml_dtypes.bfloat16),
            "biasm": biasm_host,
        }
        if s_guest:
            xg_l, wg_l, bg_l = [], [], []
            for it in plan[e]:
                if it is None:
                    xg_l.append(np.zeros((P, kb, GT), ml_dtypes.bfloat16))
                    wg_l.append(np.zeros((oh, P, kb, P), ml_dtypes.bfloat16))
                    bg_l.append(np.zeros((P, oh), np.float32))
                else:
                    ge, b, h = it
                    t0 = offs[ge] + c_main + b * GT
                    n = min(GT, offs[ge + 1] - t0)
                    xg_l.append(_tile_x(inputs[t0 : t0 + n], n_in, GT))
                    wg_l.append(
                        _tile_w(weight[ge][:, h * oh * P : (h + 1) * oh * P])
                    )
                    bg_l.append(
                        np.ascontiguousarray(
                            bias.reshape(ob, P)[h * oh : (h + 1) * oh].T
                        )
                    )
            m["xg"] = np.stack(xg_l)
            m["wg"] = np.stack(wg_l)
            m["biasg"] = np.stack(bg_l)
        in_maps.append(m)
    meta = (g, offs, plan, c_main, s_guest)
    return nc, in_maps, meta, None, (t_tokens, n_out)


def kernel(inputs, weight, bias, group_sizes):
    nc, in_maps, meta, _, (t_tokens, n_out) = _prepare(
        inputs, weight, bias, group_sizes
    )
    g, offs, plan, c_main, s_guest = meta
    oh = n_out // P // 2
    res = run_bass_kernel_spmd(nc, in_maps, core_ids=list(range(N_CORES)))

    out = np.empty((t_tokens, n_out), np.float32)
    for e in range(N_CORES):
        nm = min(int(g[e]), c_main)
        if nm:
            out[offs[e] : offs[e] + nm] = res.results[e]["ym"][:, :nm].T
        for s in range(s_guest):
            it = plan[e][s]
            if it is None:
                continue
            ge, b, h = it
            t0 = offs[ge] + c_main + b * GT
            n = min(GT, int(offs[ge + 1] - t0))
            out[t0 : t0 + n, h * oh * P : (h + 1) * oh * P] = res.results[e][
                "yg"
            ][s][:, :n].T
    return out


# revision 23
# speedup vs baseline: 1.0239x; 1.0011x over previous
"""MoE grouped linear (gmm) kernel for 8 Trainium2 NeuronCores.

Strategy (expert parallel + token load balancing, bf16 compute):
  - Tokens arrive pre-sorted by expert; group_sizes[e] tokens belong to
    expert e. Core e gets weight[e] plus up to C=512 of expert e's tokens
    (the balanced share, T/8). The "all-to-all" routing is host-side
    slicing, since kernel() sees the full inputs.
  - Excess tokens of heavy experts (g_e > C) are split into guest items of
    (<=128 tokens x one half of the output dim) and scattered one per core,
    so every core does the same 512+64-column-equivalent of PE work instead
    of padding everyone to max(g_e). Guest outputs are final values for
    their (token, out) rectangle - no cross-core reduction.
  - X and W are converted to bf16 (round-to-nearest) host-side, halving
    HBM->SBUF traffic; the PE computes bf16 x bf16 -> fp32 PSUM. Per-core
    DMA (~15.7 MB, ~44 us) then hides fully under PE time (~61 us).
  - Group 0 (o-blocks 0..3) runs k-major; its weights are packed k-major
    host-side and stream as 262 KB k-pair slices on the scalar HW-DGE ring
    while X streams on the sync ring, so the matmul stream is dense from
    the moment the HAM warmup ends. Later weights ride 1 MB pair DMAs.
  - The per-partition bias is fused into the PSUM evacuation instruction.
Host then scatters per-core main/guest outputs back to [T, Out] fp32.
"""

import numpy as np
import ml_dtypes

import concourse.bass as bass
from concourse import bacc
import concourse.mybir as mybir
import concourse.tile as tile
from concourse.bass_utils import run_bass_kernel_spmd

N_CORES = 8
P = 128
GT = 128   # guest item token width
GRP0 = 6   # o-blocks in the k-major group 0

_BUILD_CACHE: dict = {}


def _build_program(c_main: int, n_in: int, n_out: int, s_guest: int):
    kb = n_in // P   # contraction blocks
    ob = n_out // P  # output-row blocks
    oh = ob // 2     # guest o-blocks (half the output dim)
    f32 = mybir.dt.float32
    bf16 = mybir.dt.bfloat16

    nc = bacc.Bacc(
        "TRN2", target_bir_lowering=False, debug=False, num_devices=N_CORES
    )
    # X^T pre-tiled host-side: [P(k within block), kb, C] so each quarter
    # DMA is fully contiguous per partition.
    xm = nc.dram_tensor("xm", [P, kb, c_main], bf16, kind="ExternalInput")
    # Group-0 weights (o-blocks 0..GRP-1) packed k-major: [kb, P, GRP, P].
    # GRP=6 drops the k-major stream's DMA demand to ~250 KB/us, matching
    # what the SDMA path delivers while it ramps.
    GRP = GRP0
    wk0 = nc.dram_tensor("wk0", [kb, P, GRP, P], bf16, kind="ExternalInput")
    # Remaining o-slabs: [ob-GRP, P(k within block), kb, P(o)].
    wm = nc.dram_tensor("wm", [ob - GRP, P, kb, P], bf16, kind="ExternalInput")
    biasm = nc.dram_tensor("biasm", [P, ob], f32, kind="ExternalInput")
    ym = nc.dram_tensor("ym", [n_out, c_main], f32, kind="ExternalOutput")
    if s_guest:
        xg = nc.dram_tensor("xg", [s_guest, P, kb, GT], bf16, kind="ExternalInput")
        wg = nc.dram_tensor(
            "wg", [s_guest, oh, P, kb, P], bf16, kind="ExternalInput"
        )
        biasg = nc.dram_tensor("biasg", [s_guest, P, oh], f32, kind="ExternalInput")
        yg = nc.dram_tensor(
            "yg", [s_guest, oh * P, GT], f32, kind="ExternalOutput"
        )

    XQ = 2  # k-slabs per X tile (262 KB units match the wk pair slices)

    with tile.TileContext(nc) as tc:
        with (
            tc.tile_pool(name="const", bufs=1) as constp,
            tc.tile_pool(name="xtsb", bufs=1) as xtp,
            tc.tile_pool(name="wksb", bufs=6) as wkp,
            tc.tile_pool(name="wsb", bufs=(ob - GRP) // 2) as wp,
            tc.tile_pool(name="wgsb", bufs=4) as wgp,
            tc.tile_pool(name="outsb", bufs=2 * GRP) as outp,
            tc.tile_pool(name="goutsb", bufs=4) as goutp,
            tc.tile_pool(name="psum", bufs=1, space="PSUM") as psump,
        ):
            # All compute-critical loads ride the sync HW-DGE ring (the
            # scalar ring ramps ~2 us later at kernel start), interleaved
            # in exactly the k-major consumption order. The scalar ring
            # carries only the small consts, guest X, and output stores.
            # First 6 k-slabs load as singles (327 KB of W+X per slab) so
            # the semaphore granularity is fine while the SDMA path ramps;
            # the rest as pairs. wk before xp per slab: the LDWEIGHTS of a
            # slab is pulled ahead of the in-flight matmuls.
            wk_at = {}
            x_at = {}
            nsingle = min(6, kb)
            slabs = [(k, 1) for k in range(nsingle)] + [
                (k0, 2) for k0 in range(nsingle, kb, 2)
            ]
            for k0, nk in slabs:
                wt = wkp.tile([P, nk, GRP, P], bf16, tag=f"wk{nk}", name=f"wk{k0}")
                nc.sync.dma_start(
                    wt[:], wk0[k0 : k0 + nk].rearrange("k p o j -> p k o j")
                )
                xt_ = xtp.tile([P, nk, c_main], bf16, tag=f"x{k0}", name=f"x{k0}")
                nc.sync.dma_start(xt_[:], xm[:, k0 : k0 + nk, :])
                for dk in range(nk):
                    wk_at[k0 + dk] = (wt, dk)
                    x_at[k0 + dk] = (xt_, dk)
            bias_sb = constp.tile([P, ob], f32)
            nc.scalar.dma_start(bias_sb[:], biasm[:])
            if s_guest:
                biasg_sb = constp.tile([P, s_guest * oh], f32)
                nc.scalar.dma_start(
                    biasg_sb[:], biasg.rearrange("s p o -> p (s o)")
                )
                xg_sb = []
                for s in range(s_guest):
                    t = xtp.tile([P, kb, GT], bf16, tag=f"xg{s}", name=f"xg{s}")
                    nc.scalar.dma_start(t[:], xg[s])
                    xg_sb.append(t)

            # HAM warmup: dummy matmuls with no data deps run while the
            # prologue DMAs stream, so the PE clock is at 2.4 GHz (and the
            # activity window warm) when the first real matmul issues.
            warm = constp.tile([P, 512], bf16)
            nc.vector.memset(warm[:], 0)
            ps_warm = psump.tile([P, 512], f32, tag="ps0", name="warmps")
            # 8 warmup MMs (~3.4 us cold) end right when the first k-single
            # slab has landed, with the HAM activity window saturated.
            for i in range(8):
                nc.tensor.matmul(
                    ps_warm[:],
                    warm[:, :P],
                    warm[:],
                    start=(i == 0),
                    stop=(i == 7),
                )

            def evac(ps, bias_col, dst, n_cols, engine, name):
                """PSUM -> SBUF with fused per-o bias, then store. Guest
                stores ride the sync ring (idle by then), so the final
                store triggers overlap the evac engines instead of
                serializing behind them on scalar."""
                pool = outp if n_cols == c_main else goutp
                ot = pool.tile([P, n_cols], f32, tag=f"ot{n_cols}", name=name)
                if engine == 0:
                    nc.scalar.activation(
                        ot[:],
                        ps,
                        mybir.ActivationFunctionType.Identity,
                        bias=bias_col,
                    )
                else:
                    nc.vector.tensor_scalar_add(ot[:], ps, bias_col)
                if n_cols == c_main:
                    ring = nc.scalar
                else:
                    # guest stores: pick the ring whose engine is NOT doing
                    # this evac, so the final store triggers run in parallel
                    ring = nc.sync if engine == 0 else nc.scalar
                ring.dma_start(dst, ot[:])

            # All accumulators come from one strict round-robin over GRP
            # full PSUM banks: every bank's next user is GRP o-blocks
            # behind its evacuation, so no phase stalls on a bank. Guests
            # write only the first GT columns of a full-bank tile.
            ps_seq = [0]

            def next_ps(name):
                t = psump.tile(
                    [P, c_main], f32, tag=f"ps{ps_seq[0] % GRP}", name=name
                )
                ps_seq[0] += 1
                return t

            # Group 0 (o-blocks 0..GRP-1) runs k-major so every arriving
            # X-slab immediately feeds GRP o-columns of PE work.
            g0 = list(range(GRP))
            pss0 = {oi: next_ps(f"ps{oi}") for oi in g0}
            for k in range(kb):
                wt, wi = wk_at[k]
                xt_, xi = x_at[k]
                for oi in g0:
                    nc.tensor.matmul(
                        pss0[oi][:],
                        wt[:, wi, oi, :],
                        xt_[:, xi, :],
                        start=(k == 0),
                        stop=(k == kb - 1),
                    )
            for oi in g0:
                evac(
                    pss0[oi][:], bias_sb[:, oi : oi + 1],
                    ym[oi * P : (oi + 1) * P, :], c_main, oi % 2, f"ot{oi}",
                )

            # Remaining o-blocks run one at a time off 1 MB pair DMAs:
            # per-bank k-runs rotate through the PSUM banks (released by
            # group 0 in the same order), and evacuations pipeline under
            # the next bank's MMs.
            for pi in range((ob - GRP) // 2):
                wpair = wp.tile([P, 2, kb, P], bf16, tag="wo", name=f"wp{pi}")
                nc.sync.dma_start(
                    wpair[:],
                    wm[2 * pi : 2 * pi + 2].rearrange("o p k j -> p o k j"),
                )
                for j in (0, 1):
                    o = GRP + 2 * pi + j
                    ps = next_ps(f"ps{o}")
                    for k in range(kb):
                        xt_, xi = x_at[k]
                        nc.tensor.matmul(
                            ps[:],
                            wpair[:, j, k, :],
                            xt_[:, xi, :],
                            start=(k == 0),
                            stop=(k == kb - 1),
                        )
                    evac(
                        ps[:], bias_sb[:, o : o + 1],
                        ym[o * P : (o + 1) * P, :], c_main, o % 2, f"ot{o}",
                    )

            # Guest pass: s_guest items of (GT tokens x oh o-blocks), final
            # outputs for their rectangle (no cross-core reduction).
            if s_guest:
                for s in range(s_guest):
                    for gp in range(oh // 2):
                        wgpair = wgp.tile(
                            [P, 2, kb, P], bf16, tag="wg", name=f"wg{s}_{gp}"
                        )
                        nc.sync.dma_start(
                            wgpair[:],
                            wg[s, 2 * gp : 2 * gp + 2].rearrange(
                                "o p k j -> p o k j"
                            ),
                        )
                        for j in (0, 1):
                            o = 2 * gp + j
                            ps = next_ps(f"psg{s}_{o}")
                            for k in range(kb):
                                nc.tensor.matmul(
                                    ps[:, :GT],
                                    wgpair[:, j, k, :],
                                    xg_sb[s][:, k, :],
                                    start=(k == 0),
                                    stop=(k == kb - 1),
                                )
                            evac(
                                ps[:, :GT],
                                biasg_sb[:, s * oh + o : s * oh + o + 1],
                                yg[s, o * P : (o + 1) * P, :], GT, o % 2,
                                f"go{s}_{o}",
                            )
    nc.finalize()
    return nc


def _tile_x(x_cols: np.ndarray, n_in: int, width: int) -> np.ndarray:
    """[n, n_in] fp32 token rows -> [P, kb, width] bf16 X^T tiling."""
    kb = n_in // P
    xt = np.zeros((n_in, width), np.float32)
    xt[:, : x_cols.shape[0]] = x_cols.T
    return np.ascontiguousarray(
        xt.reshape(kb, P, width).transpose(1, 0, 2)
    ).astype(ml_dtypes.bfloat16)


def _tile_w(w: np.ndarray) -> np.ndarray:
    """[n_in, n_o] fp32 -> [n_o//P, P, n_in//P, P] bf16 o-slab tiling."""
    n_in, n_o = w.shape
    return np.ascontiguousarray(
        w.reshape(n_in // P, P, n_o // P, P).transpose(2, 1, 0, 3)
    ).astype(ml_dtypes.bfloat16)


def _prepare(inputs, weight, bias, group_sizes):
    """Build (or reuse) the program and the per-core input maps."""
    inputs = np.ascontiguousarray(np.asarray(inputs, dtype=np.float32))
    weight = np.ascontiguousarray(np.asarray(weight, dtype=np.float32))
    bias = np.ascontiguousarray(np.asarray(bias, dtype=np.float32))
    g = np.asarray(group_sizes).astype(np.int64)

    t_tokens, n_in = inputs.shape
    n_exp, _, n_out = weight.shape
    assert n_exp == N_CORES, f"expected {N_CORES} experts, got {n_exp}"
    offs = np.concatenate([[0], np.cumsum(g)])
    assert offs[-1] == t_tokens, "group_sizes must sum to token count"

    kb, ob, oh = n_in // P, n_out // P, n_out // P // 2
    GRP = GRP0
    c_main = max(P, -(-t_tokens // N_CORES // P) * P)  # balanced share

    # Guest items: (expert, token-block, out-half) for tokens beyond c_main.
    items = []
    for e in range(n_exp):
        nblk = -(-max(0, int(g[e]) - c_main) // GT)
        for b in range(nblk):
            for h in range(2):
                items.append((e, b, h))
    s_guest = -(-len(items) // N_CORES) if items else 0

    key = (c_main, n_in, n_out, s_guest)
    if key not in _BUILD_CACHE:
        _BUILD_CACHE[key] = _build_program(c_main, n_in, n_out, s_guest)
    nc = _BUILD_CACHE[key]

    biasm_host = np.ascontiguousarray(bias.reshape(ob, P).T)  # [P, ob]

    # slot-major assignment: item i -> (core i % 8, slot i // 8)
    plan = [[None] * s_guest for _ in range(n_exp)]
    for i, it in enumerate(items):
        plan[i % N_CORES][i // N_CORES] = it

    in_maps = []
    for e in range(n_exp):
        nm = min(int(g[e]), c_main)
        wt = weight[e].reshape(kb, P, ob, P)
        m = {
            "xm": _tile_x(inputs[offs[e] : offs[e] + nm], n_in, c_main),
            "wk0": np.ascontiguousarray(wt[:, :, :GRP, :]).astype(
                ml_dtypes.bfloat16
            ),
            "wm": np.ascontiguousarray(
                wt[:, :, GRP:, :].transpose(2, 1, 0, 3)
            ).astype(ml_dtypes.bfloat16),
            "biasm": biasm_host,
        }
        if s_guest:
            xg_l, wg_l, bg_l = [], [], []
            for it in plan[e]:
                if it is None:
                    xg_l.append(np.zeros((P, kb, GT), ml_dtypes.bfloat16))
                    wg_l.append(np.zeros((oh, P, kb, P), ml_dtypes.bfloat16))
                    bg_l.append(np.zeros((P, oh), np.float32))
                else:
                    ge, b, h = it
                    t0 = offs[ge] + c_main + b * GT
                    n = min(GT, offs[ge + 1] - t0)
                    xg_l.append(_tile_x(inputs[t0 : t0 + n], n_in, GT))
                    wg_l.append(
                        _tile_w(weight[ge][:, h * oh * P : (h + 1) * oh * P])
                    )
                    bg_l.append(
                        np.ascontiguousarray(
                            bias.reshape(ob, P)[h * oh : (h + 1) * oh].T
                        )
                    )
            m["xg"] = np.stack(xg_l)
            m["wg"] = np.stack(wg_l)
            m["biasg"] = np.stack(bg_l)
        in_maps.append(m)
    meta = (g, offs, plan, c_main, s_guest)
    return nc, in_maps, meta, None, (t_tokens, n_out)


def kernel(inputs, weight, bias, group_sizes):
    nc, in_maps, meta, _, (t_tokens, n_out) = _prepare(
        inputs, weight, bias, group_sizes
    )
    g, offs, plan, c_main, s_guest = meta
    oh = n_out // P // 2
    res = run_bass_kernel_spmd(nc, in_maps, core_ids=list(range(N_CORES)))

    out = np.empty((t_tokens, n_out), np.float32)
    for e in range(N_CORES):
        nm = min(int(g[e]), c_main)
        if nm:
            out[offs[e] : offs[e] + nm] = res.results[e]["ym"][:, :nm].T
        for s in range(s_guest):
            it = plan[e][s]
            if it is None:
                continue
            ge, b, h = it
            t0 = offs[ge] + c_main + b * GT
            n = min(GT, int(offs[ge + 1] - t0))
            out[t0 : t0 + n, h * oh * P : (h + 1) * oh * P] = res.results[e][
                "yg"
            ][s][:, :n].T
    return out


# revision 25
# speedup vs baseline: 1.0285x; 1.0045x over previous
"""MoE grouped linear (gmm) kernel for 8 Trainium2 NeuronCores.

Strategy (expert parallel + token load balancing, bf16 compute):
  - Tokens arrive pre-sorted by expert; group_sizes[e] tokens belong to
    expert e. Core e gets weight[e] plus up to C=512 of expert e's tokens
    (the balanced share, T/8). The "all-to-all" routing is host-side
    slicing, since kernel() sees the full inputs.
  - Excess tokens of heavy experts (g_e > C) are split into guest items of
    (<=128 tokens x one half of the output dim) and scattered one per core,
    so every core does the same 512+64-column-equivalent of PE work instead
    of padding everyone to max(g_e). Guest outputs are final values for
    their (token, out) rectangle - no cross-core reduction.
  - X and W are converted to bf16 (round-to-nearest) host-side, halving
    HBM->SBUF traffic; the PE computes bf16 x bf16 -> fp32 PSUM. Per-core
    DMA (~15.7 MB, ~44 us) then hides fully under PE time (~61 us).
  - Group 0 (o-blocks 0..3) runs k-major; its weights are packed k-major
    host-side and stream as 262 KB k-pair slices on the scalar HW-DGE ring
    while X streams on the sync ring, so the matmul stream is dense from
    the moment the HAM warmup ends. Later weights ride 1 MB pair DMAs.
  - The per-partition bias is fused into the PSUM evacuation instruction.
Host then scatters per-core main/guest outputs back to [T, Out] fp32.
"""

import numpy as np
import ml_dtypes

import concourse.bass as bass
from concourse import bacc
import concourse.mybir as mybir
import concourse.tile as tile
from concourse.bass_utils import run_bass_kernel_spmd

N_CORES = 8
P = 128
GT = 128  # guest item token width

_BUILD_CACHE: dict = {}


def _build_program(c_main: int, n_in: int, n_out: int, s_guest: int):
    kb = n_in // P   # contraction blocks
    ob = n_out // P  # output-row blocks
    oh = ob // 2     # guest o-blocks (half the output dim)
    f32 = mybir.dt.float32
    bf16 = mybir.dt.bfloat16

    nc = bacc.Bacc(
        "TRN2", target_bir_lowering=False, debug=False, num_devices=N_CORES
    )
    # X^T pre-tiled host-side: [P(k within block), kb, C] so each quarter
    # DMA is fully contiguous per partition.
    xm = nc.dram_tensor("xm", [P, kb, c_main], bf16, kind="ExternalInput")
    # Group-0 weights (o-blocks 0..GRP-1) packed k-major: [kb, P, GRP, P].
    GRP = 4
    wk0 = nc.dram_tensor("wk0", [kb, P, GRP, P], bf16, kind="ExternalInput")
    # Remaining o-slabs: [ob-GRP, P(k within block), kb, P(o)].
    wm = nc.dram_tensor("wm", [ob - GRP, P, kb, P], bf16, kind="ExternalInput")
    biasm = nc.dram_tensor("biasm", [P, ob], f32, kind="ExternalInput")
    # Outputs travel as bf16 (halves store traffic; host upcasts).
    ym = nc.dram_tensor("ym", [n_out, c_main], bf16, kind="ExternalOutput")
    if s_guest:
        xg = nc.dram_tensor("xg", [s_guest, P, kb, GT], bf16, kind="ExternalInput")
        wg = nc.dram_tensor(
            "wg", [s_guest, oh, P, kb, P], bf16, kind="ExternalInput"
        )
        biasg = nc.dram_tensor("biasg", [s_guest, P, oh], f32, kind="ExternalInput")
        yg = nc.dram_tensor(
            "yg", [s_guest, oh * P, GT], bf16, kind="ExternalOutput"
        )

    XQ = 2  # k-slabs per X tile (262 KB units match the wk pair slices)

    with tile.TileContext(nc) as tc:
        with (
            tc.tile_pool(name="const", bufs=1) as constp,
            tc.tile_pool(name="xtsb", bufs=1) as xtp,
            tc.tile_pool(name="wksb", bufs=kb // 2) as wkp,
            tc.tile_pool(name="wsb", bufs=(ob - GRP) // 2) as wp,
            tc.tile_pool(name="wgsb", bufs=4) as wgp,
            tc.tile_pool(name="outsb", bufs=2 * GRP) as outp,
            tc.tile_pool(name="goutsb", bufs=4) as goutp,
            tc.tile_pool(name="psum", bufs=1, space="PSUM") as psump,
        ):
            # All compute-critical loads ride the sync HW-DGE ring (the
            # scalar ring ramps ~2 us later at kernel start), interleaved
            # in exactly the k-major consumption order. The scalar ring
            # carries only the small consts, guest X, and output stores.
            wks = [None] * (kb // 2)
            xqs = [None] * (kb // XQ)

            def load_wk(q):
                t = wkp.tile([P, 2, GRP, P], bf16, tag="wk", name=f"wk{q}")
                nc.sync.dma_start(
                    t[:], wk0[2 * q : 2 * q + 2].rearrange("k p o j -> p k o j")
                )
                wks[q] = t

            def load_xq(i):
                t = xtp.tile([P, XQ, c_main], bf16, tag=f"xq{i}", name=f"xq{i}")
                nc.sync.dma_start(t[:], xm[:, i * XQ : (i + 1) * XQ, :])
                xqs[i] = t

            for q in range(kb // 2):
                load_wk(q)
                if q < kb // XQ:
                    load_xq(q)
            bias_sb = constp.tile([P, ob], f32)
            nc.scalar.dma_start(bias_sb[:], biasm[:])
            if s_guest:
                biasg_sb = constp.tile([P, s_guest * oh], f32)
                nc.scalar.dma_start(
                    biasg_sb[:], biasg.rearrange("s p o -> p (s o)")
                )
                xg_sb = []
                for s in range(s_guest):
                    t = xtp.tile([P, kb, GT], bf16, tag=f"xg{s}", name=f"xg{s}")
                    nc.scalar.dma_start(t[:], xg[s])
                    xg_sb.append(t)

            # HAM warmup: dummy matmuls with no data deps run while the
            # prologue DMAs stream, so the PE clock is at 2.4 GHz (and the
            # activity window warm) when the first real matmul issues.
            warm = constp.tile([P, 512], bf16)
            nc.vector.memset(warm[:], 0)
            ps_warm = psump.tile([P, 512], f32, tag="ps0", name="warmps")
            for i in range(8):
                nc.tensor.matmul(
                    ps_warm[:],
                    warm[:, :P],
                    warm[:],
                    start=(i == 0),
                    stop=(i == 7),
                )

            def evac(ps, bias_col, dst, n_cols, engine, name):
                """PSUM -> SBUF with fused per-o bias, then store. Guest
                stores ride the sync ring (idle by then), so the final
                store triggers overlap the evac engines instead of
                serializing behind them on scalar."""
                pool = outp if n_cols == c_main else goutp
                ot = pool.tile([P, n_cols], bf16, tag=f"ot{n_cols}", name=name)
                if engine == 0:
                    nc.scalar.activation(
                        ot[:],
                        ps[:],
                        mybir.ActivationFunctionType.Identity,
                        bias=bias_col,
                    )
                else:
                    nc.vector.tensor_scalar_add(ot[:], ps[:], bias_col)
                if n_cols == c_main:
                    ring = nc.scalar
                else:
                    # guest stores: pick the ring whose engine is NOT doing
                    # this evac, so the final store triggers run in parallel
                    ring = nc.sync if engine == 0 else nc.scalar
                ring.dma_start(dst, ot[:])

            # Group 0 (o-blocks 0..GRP-1) runs k-major so every arriving
            # X-slab immediately feeds GRP o-columns of PE work.
            g0 = list(range(GRP))
            pss0 = {
                oi: psump.tile([P, c_main], f32, tag=f"ps{oi}", name=f"ps{oi}")
                for oi in g0
            }
            for k in range(kb):
                xvk = xqs[k // XQ][:, k % XQ, :]
                wvk = wks[k // 2]
                for oi in g0:
                    nc.tensor.matmul(
                        pss0[oi][:],
                        wvk[:, k % 2, oi, :],
                        xvk,
                        start=(k == 0),
                        stop=(k == kb - 1),
                    )
            for oi in g0:
                evac(
                    pss0[oi], bias_sb[:, oi : oi + 1],
                    ym[oi * P : (oi + 1) * P, :], c_main, oi % 2, f"ot{oi}",
                )

            # Remaining o-blocks run one at a time off 1 MB pair DMAs:
            # per-bank k-runs rotate through the PSUM banks (released by
            # group 0 in the same order), and evacuations pipeline under
            # the next bank's MMs.
            for pi in range((ob - GRP) // 2):
                wpair = wp.tile([P, 2, kb, P], bf16, tag="wo", name=f"wp{pi}")
                nc.sync.dma_start(
                    wpair[:],
                    wm[2 * pi : 2 * pi + 2].rearrange("o p k j -> p o k j"),
                )
                for j in (0, 1):
                    o = GRP + 2 * pi + j
                    # 5-bank rotation starting on the fresh ps4 bank, so o4
                    # does not wait for group 0's first evacuation.
                    ps = psump.tile(
                        [P, c_main], f32,
                        tag=f"ps{(o - GRP + 4) % 5}", name=f"ps{o}",
                    )
                    for k in range(kb):
                        nc.tensor.matmul(
                            ps[:],
                            wpair[:, j, k, :],
                            xqs[k // XQ][:, k % XQ, :],
                            start=(k == 0),
                            stop=(k == kb - 1),
                        )
                    evac(
                        ps, bias_sb[:, o : o + 1],
                        ym[o * P : (o + 1) * P, :], c_main, o % 2, f"ot{o}",
                    )

            # Guest pass: s_guest items of (GT tokens x oh o-blocks), final
            # outputs for their rectangle (no cross-core reduction).
            if s_guest:
                for s in range(s_guest):
                    for gp in range(oh // 2):
                        wgpair = wgp.tile(
                            [P, 2, kb, P], bf16, tag="wg", name=f"wg{s}_{gp}"
                        )
                        nc.sync.dma_start(
                            wgpair[:],
                            wg[s, 2 * gp : 2 * gp + 2].rearrange(
                                "o p k j -> p o k j"
                            ),
                        )
                        for j in (0, 1):
                            o = 2 * gp + j
                            ps = psump.tile(
                                [P, GT], f32, tag=f"psg{o % 3}",
                                name=f"psg{s}_{o}",
                            )
                            for k in range(kb):
                                nc.tensor.matmul(
                                    ps[:],
                                    wgpair[:, j, k, :],
                                    xg_sb[s][:, k, :],
                                    start=(k == 0),
                                    stop=(k == kb - 1),
                                )
                            evac(
                                ps,
                                biasg_sb[:, s * oh + o : s * oh + o + 1],
                                yg[s, o * P : (o + 1) * P, :], GT, o % 2,
                                f"go{s}_{o}",
                            )
    nc.finalize()
    return nc


def _tile_x(x_cols: np.ndarray, n_in: int, width: int) -> np.ndarray:
    """[n, n_in] fp32 token rows -> [P, kb, width] bf16 X^T tiling."""
    kb = n_in // P
    xt = np.zeros((n_in, width), np.float32)
    xt[:, : x_cols.shape[0]] = x_cols.T
    return np.ascontiguousarray(
        xt.reshape(kb, P, width).transpose(1, 0, 2)
    ).astype(ml_dtypes.bfloat16)


def _tile_w(w: np.ndarray) -> np.ndarray:
    """[n_in, n_o] fp32 -> [n_o//P, P, n_in//P, P] bf16 o-slab tiling."""
    n_in, n_o = w.shape
    return np.ascontiguousarray(
        w.reshape(n_in // P, P, n_o // P, P).transpose(2, 1, 0, 3)
    ).astype(ml_dtypes.bfloat16)


def _prepare(inputs, weight, bias, group_sizes):
    """Build (or reuse) the program and the per-core input maps."""
    inputs = np.ascontiguousarray(np.asarray(inputs, dtype=np.float32))
    weight = np.ascontiguousarray(np.asarray(weight, dtype=np.float32))
    bias = np.ascontiguousarray(np.asarray(bias, dtype=np.float32))
    g = np.asarray(group_sizes).astype(np.int64)

    t_tokens, n_in = inputs.shape
    n_exp, _, n_out = weight.shape
    assert n_exp == N_CORES, f"expected {N_CORES} experts, got {n_exp}"
    offs = np.concatenate([[0], np.cumsum(g)])
    assert offs[-1] == t_tokens, "group_sizes must sum to token count"

    kb, ob, oh = n_in // P, n_out // P, n_out // P // 2
    GRP = 4
    c_main = max(P, -(-t_tokens // N_CORES // P) * P)  # balanced share

    # Guest items: (expert, token-block, out-half) for tokens beyond c_main.
    items = []
    for e in range(n_exp):
        nblk = -(-max(0, int(g[e]) - c_main) // GT)
        for b in range(nblk):
            for h in range(2):
                items.append((e, b, h))
    s_guest = -(-len(items) // N_CORES) if items else 0

    key = (c_main, n_in, n_out, s_guest)
    if key not in _BUILD_CACHE:
        _BUILD_CACHE[key] = _build_program(c_main, n_in, n_out, s_guest)
    nc = _BUILD_CACHE[key]

    biasm_host = np.ascontiguousarray(bias.reshape(ob, P).T)  # [P, ob]

    # slot-major assignment: item i -> (core i % 8, slot i // 8)
    plan = [[None] * s_guest for _ in range(n_exp)]
    for i, it in enumerate(items):
        plan[i % N_CORES][i // N_CORES] = it

    in_maps = []
    for e in range(n_exp):
        nm = min(int(g[e]), c_main)
        wt = weight[e].reshape(kb, P, ob, P)
        m = {
            "xm": _tile_x(inputs[offs[e] : offs[e] + nm], n_in, c_main),
            "wk0": np.ascontiguousarray(wt[:, :, :GRP, :]).astype(
                ml_dtypes.bfloat16
            ),
            "wm": np.ascontiguousarray(
                wt[:, :, GRP:, :].transpose(2, 1, 0, 3)
            ).astype(ml_dtypes.bfloat16),
            "biasm": biasm_host,
        }
        if s_guest:
            xg_l, wg_l, bg_l = [], [], []
            for it in plan[e]:
                if it is None:
                    xg_l.append(np.zeros((P, kb, GT), ml_dtypes.bfloat16))
                    wg_l.append(np.zeros((oh, P, kb, P), ml_dtypes.bfloat16))
                    bg_l.append(np.zeros((P, oh), np.float32))
                else:
                    ge, b, h = it
                    t0 = offs[ge] + c_main + b * GT
                    n = min(GT, offs[ge + 1] - t0)
                    xg_l.append(_tile_x(inputs[t0 : t0 + n], n_in, GT))
                    wg_l.append(
                        _tile_w(weight[ge][:, h * oh * P : (h + 1) * oh * P])
                    )
                    bg_l.append(
                        np.ascontiguousarray(
                            bias.reshape(ob, P)[h * oh : (h + 1) * oh].T
                        )
                    )
            m["xg"] = np.stack(xg_l)
            m["wg"] = np.stack(wg_l)
            m["biasg"] = np.stack(bg_l)
        in_maps.append(m)
    meta = (g, offs, plan, c_main, s_guest)
    return nc, in_maps, meta, None, (t_tokens, n_out)


def kernel(inputs, weight, bias, group_sizes):
    nc, in_maps, meta, _, (t_tokens, n_out) = _prepare(
        inputs, weight, bias, group_sizes
    )
    g, offs, plan, c_main, s_guest = meta
    oh = n_out // P // 2
    res = run_bass_kernel_spmd(nc, in_maps, core_ids=list(range(N_CORES)))

    out = np.empty((t_tokens, n_out), np.float32)
    for e in range(N_CORES):
        nm = min(int(g[e]), c_main)
        if nm:
            out[offs[e] : offs[e] + nm] = (
                res.results[e]["ym"][:, :nm].astype(np.float32).T
            )
        for s in range(s_guest):
            it = plan[e][s]
            if it is None:
                continue
            ge, b, h = it
            t0 = offs[ge] + c_main + b * GT
            n = min(GT, int(offs[ge + 1] - t0))
            out[t0 : t0 + n, h * oh * P : (h + 1) * oh * P] = (
                res.results[e]["yg"][s][:, :n].astype(np.float32).T
            )
    return out


# revision 26
# speedup vs baseline: 1.0464x; 1.0174x over previous
"""MoE grouped linear (gmm) kernel for 8 Trainium2 NeuronCores.

Strategy (expert parallel + token load balancing, bf16 compute):
  - Tokens arrive pre-sorted by expert; group_sizes[e] tokens belong to
    expert e. Core e gets weight[e] plus up to C=512 of expert e's tokens
    (the balanced share, T/8). The "all-to-all" routing is host-side
    slicing, since kernel() sees the full inputs.
  - Excess tokens of heavy experts (g_e > C) are split into guest items of
    (<=128 tokens x one half of the output dim) and scattered one per core,
    so every core does the same 512+64-column-equivalent of PE work instead
    of padding everyone to max(g_e). Guest outputs are final values for
    their (token, out) rectangle - no cross-core reduction.
  - X and W are converted to bf16 (round-to-nearest) host-side, halving
    HBM->SBUF traffic; the PE computes bf16 x bf16 -> fp32 PSUM. Per-core
    DMA (~15.7 MB, ~44 us) then hides fully under PE time (~61 us).
  - Group 0 (o-blocks 0..3) runs k-major; its weights are packed k-major
    host-side and stream as 262 KB k-pair slices on the scalar HW-DGE ring
    while X streams on the sync ring, so the matmul stream is dense from
    the moment the HAM warmup ends. Later weights ride 1 MB pair DMAs.
  - The per-partition bias is fused into the PSUM evacuation instruction.
Host then scatters per-core main/guest outputs back to [T, Out] fp32.
"""

import numpy as np
import ml_dtypes

import concourse.bass as bass
from concourse import bacc
import concourse.mybir as mybir
import concourse.tile as tile
from concourse.bass_utils import run_bass_kernel_spmd

N_CORES = 8
P = 128
GT = 128   # guest item token width
GRP0 = 8   # o-blocks in the k-major group 0

_BUILD_CACHE: dict = {}


def _build_program(c_main: int, n_in: int, n_out: int, s_guest: int):
    kb = n_in // P   # contraction blocks
    ob = n_out // P  # output-row blocks
    oh = ob // 2     # guest o-blocks (half the output dim)
    f32 = mybir.dt.float32
    bf16 = mybir.dt.bfloat16

    nc = bacc.Bacc(
        "TRN2", target_bir_lowering=False, debug=False, num_devices=N_CORES
    )
    # X^T pre-tiled host-side: [P(k within block), kb, C] so each quarter
    # DMA is fully contiguous per partition.
    xm = nc.dram_tensor("xm", [P, kb, c_main], bf16, kind="ExternalInput")
    # Group-0 weights (o-blocks 0..GRP-1) packed k-major: [kb, P, GRP, P].
    # GRP=8 drops the k-major stream's DMA demand rate to ~225 KB/us
    # (rate = 76800/N + 600/GRP), matching the measured early-SDMA supply,
    # with k-single 262 KB units keeping semaphore granularity fine.
    GRP = GRP0
    wk0 = nc.dram_tensor("wk0", [kb, P, GRP, P], bf16, kind="ExternalInput")
    # Remaining o-slabs: [ob-GRP, P(k within block), kb, P(o)].
    wm = nc.dram_tensor("wm", [ob - GRP, P, kb, P], bf16, kind="ExternalInput")
    biasm = nc.dram_tensor("biasm", [P, ob], f32, kind="ExternalInput")
    # Outputs travel as bf16 (halves store traffic; host upcasts).
    ym = nc.dram_tensor("ym", [n_out, c_main], bf16, kind="ExternalOutput")
    if s_guest:
        xg = nc.dram_tensor("xg", [s_guest, P, kb, GT], bf16, kind="ExternalInput")
        wg = nc.dram_tensor(
            "wg", [s_guest, oh, P, kb, P], bf16, kind="ExternalInput"
        )
        biasg = nc.dram_tensor("biasg", [s_guest, P, oh], f32, kind="ExternalInput")
        yg = nc.dram_tensor(
            "yg", [s_guest, oh * P, GT], bf16, kind="ExternalOutput"
        )

    XQ = 2  # k-slabs per X tile (262 KB units match the wk pair slices)

    with tile.TileContext(nc) as tc:
        with (
            tc.tile_pool(name="const", bufs=1) as constp,
            tc.tile_pool(name="xtsb", bufs=1) as xtp,
            tc.tile_pool(name="wksb", bufs=kb) as wkp,
            tc.tile_pool(name="wsb", bufs=(ob - GRP) // 2) as wp,
            tc.tile_pool(name="wgsb", bufs=4) as wgp,
            tc.tile_pool(name="outsb", bufs=2 * GRP) as outp,
            tc.tile_pool(name="goutsb", bufs=4) as goutp,
            tc.tile_pool(name="psum", bufs=1, space="PSUM") as psump,
        ):
            # All compute-critical loads ride the sync HW-DGE ring (the
            # scalar ring ramps ~2 us later at kernel start), interleaved
            # in exactly the k-major consumption order. The scalar ring
            # carries only the small consts, guest X, and output stores.
            wks = [None] * kb
            xqs = [None] * kb
            for k in range(kb):
                wt = wkp.tile([P, GRP, P], bf16, tag="wk", name=f"wk{k}")
                nc.sync.dma_start(
                    wt[:], wk0[k : k + 1].rearrange("k p o j -> p (k o) j")
                )
                wks[k] = wt
                xt_ = xtp.tile([P, c_main], bf16, tag=f"x{k}", name=f"x{k}")
                nc.sync.dma_start(xt_[:], xm[:, k, :])
                xqs[k] = xt_
            bias_sb = constp.tile([P, ob], f32)
            nc.scalar.dma_start(bias_sb[:], biasm[:])
            if s_guest:
                biasg_sb = constp.tile([P, s_guest * oh], f32)
                nc.scalar.dma_start(
                    biasg_sb[:], biasg.rearrange("s p o -> p (s o)")
                )
                xg_sb = []
                for s in range(s_guest):
                    t = xtp.tile([P, kb, GT], bf16, tag=f"xg{s}", name=f"xg{s}")
                    nc.scalar.dma_start(t[:], xg[s])
                    xg_sb.append(t)

            # HAM warmup: dummy matmuls with no data deps run while the
            # prologue DMAs stream, so the PE clock is at 2.4 GHz (and the
            # activity window warm) when the first real matmul issues.
            warm = constp.tile([P, 512], bf16)
            nc.vector.memset(warm[:], 0)
            ps_warm = psump.tile([P, 512], f32, tag="ps0", name="warmps")
            for i in range(8):
                nc.tensor.matmul(
                    ps_warm[:],
                    warm[:, :P],
                    warm[:],
                    start=(i == 0),
                    stop=(i == 7),
                )

            def evac(ps, bias_col, dst, n_cols, engine, name):
                """PSUM -> SBUF with fused per-o bias, then store. Guest
                stores ride the sync ring (idle by then), so the final
                store triggers overlap the evac engines instead of
                serializing behind them on scalar."""
                pool = outp if n_cols == c_main else goutp
                ot = pool.tile([P, n_cols], bf16, tag=f"ot{n_cols}", name=name)
                if engine == 0:
                    nc.scalar.activation(
                        ot[:],
                        ps,
                        mybir.ActivationFunctionType.Identity,
                        bias=bias_col,
                    )
                else:
                    nc.vector.tensor_scalar_add(ot[:], ps, bias_col)
                if n_cols == c_main:
                    ring = nc.scalar
                else:
                    # guest stores: pick the ring whose engine is NOT doing
                    # this evac, so the final store triggers run in parallel
                    ring = nc.sync if engine == 0 else nc.scalar
                ring.dma_start(dst, ot[:])

            # One strict round-robin over all 8 PSUM banks; every bank's
            # next user trails its evacuation by 8 o-blocks. Guests write
            # only the first GT columns of a full-bank tile.
            ps_seq = [0]

            def next_ps(name):
                t = psump.tile(
                    [P, c_main], f32, tag=f"ps{ps_seq[0] % GRP}", name=name
                )
                ps_seq[0] += 1
                return t

            # Group 0 (o-blocks 0..GRP-1) runs k-major so every arriving
            # X-slab immediately feeds GRP o-columns of PE work.
            g0 = list(range(GRP))
            pss0 = {oi: next_ps(f"ps{oi}") for oi in g0}
            for k in range(kb):
                for oi in g0:
                    nc.tensor.matmul(
                        pss0[oi][:],
                        wks[k][:, oi, :],
                        xqs[k][:],
                        start=(k == 0),
                        stop=(k == kb - 1),
                    )
            for oi in g0:
                evac(
                    pss0[oi][:], bias_sb[:, oi : oi + 1],
                    ym[oi * P : (oi + 1) * P, :], c_main, oi % 2, f"ot{oi}",
                )

            # Remaining o-blocks run one at a time off 1 MB pair DMAs:
            # per-bank k-runs rotate through the PSUM banks (released by
            # group 0 in the same order), and evacuations pipeline under
            # the next bank's MMs.
            for pi in range((ob - GRP) // 2):
                wpair = wp.tile([P, 2, kb, P], bf16, tag="wo", name=f"wp{pi}")
                nc.sync.dma_start(
                    wpair[:],
                    wm[2 * pi : 2 * pi + 2].rearrange("o p k j -> p o k j"),
                )
                for j in (0, 1):
                    o = GRP + 2 * pi + j
                    ps = next_ps(f"ps{o}")
                    for k in range(kb):
                        nc.tensor.matmul(
                            ps[:],
                            wpair[:, j, k, :],
                            xqs[k][:],
                            start=(k == 0),
                            stop=(k == kb - 1),
                        )
                    evac(
                        ps[:], bias_sb[:, o : o + 1],
                        ym[o * P : (o + 1) * P, :], c_main, o % 2, f"ot{o}",
                    )

            # Guest pass: s_guest items of (GT tokens x oh o-blocks), final
            # outputs for their rectangle (no cross-core reduction).
            if s_guest:
                for s in range(s_guest):
                    for gp in range(oh // 2):
                        wgpair = wgp.tile(
                            [P, 2, kb, P], bf16, tag="wg", name=f"wg{s}_{gp}"
                        )
                        nc.sync.dma_start(
                            wgpair[:],
                            wg[s, 2 * gp : 2 * gp + 2].rearrange(
                                "o p k j -> p o k j"
                            ),
                        )
                        for j in (0, 1):
                            o = 2 * gp + j
                            ps = next_ps(f"psg{s}_{o}")
                            for k in range(kb):
                                nc.tensor.matmul(
                                    ps[:, :GT],
                                    wgpair[:, j, k, :],
                                    xg_sb[s][:, k, :],
                                    start=(k == 0),
                                    stop=(k == kb - 1),
                                )
                            evac(
                                ps[:, :GT],
                                biasg_sb[:, s * oh + o : s * oh + o + 1],
                                yg[s, o * P : (o + 1) * P, :], GT, o % 2,
                                f"go{s}_{o}",
                            )
    nc.finalize()
    return nc


def _tile_x(x_cols: np.ndarray, n_in: int, width: int) -> np.ndarray:
    """[n, n_in] fp32 token rows -> [P, kb, width] bf16 X^T tiling."""
    kb = n_in // P
    xt = np.zeros((n_in, width), np.float32)
    xt[:, : x_cols.shape[0]] = x_cols.T
    return np.ascontiguousarray(
        xt.reshape(kb, P, width).transpose(1, 0, 2)
    ).astype(ml_dtypes.bfloat16)


def _tile_w(w: np.ndarray) -> np.ndarray:
    """[n_in, n_o] fp32 -> [n_o//P, P, n_in//P, P] bf16 o-slab tiling."""
    n_in, n_o = w.shape
    return np.ascontiguousarray(
        w.reshape(n_in // P, P, n_o // P, P).transpose(2, 1, 0, 3)
    ).astype(ml_dtypes.bfloat16)


def _prepare(inputs, weight, bias, group_sizes):
    """Build (or reuse) the program and the per-core input maps."""
    inputs = np.ascontiguousarray(np.asarray(inputs, dtype=np.float32))
    weight = np.ascontiguousarray(np.asarray(weight, dtype=np.float32))
    bias = np.ascontiguousarray(np.asarray(bias, dtype=np.float32))
    g = np.asarray(group_sizes).astype(np.int64)

    t_tokens, n_in = inputs.shape
    n_exp, _, n_out = weight.shape
    assert n_exp == N_CORES, f"expected {N_CORES} experts, got {n_exp}"
    offs = np.concatenate([[0], np.cumsum(g)])
    assert offs[-1] == t_tokens, "group_sizes must sum to token count"

    kb, ob, oh = n_in // P, n_out // P, n_out // P // 2
    GRP = GRP0
    c_main = max(P, -(-t_tokens // N_CORES // P) * P)  # balanced share

    # Guest items: (expert, token-block, out-half) for tokens beyond c_main.
    items = []
    for e in range(n_exp):
        nblk = -(-max(0, int(g[e]) - c_main) // GT)
        for b in range(nblk):
            for h in range(2):
                items.append((e, b, h))
    s_guest = -(-len(items) // N_CORES) if items else 0

    key = (c_main, n_in, n_out, s_guest)
    if key not in _BUILD_CACHE:
        _BUILD_CACHE[key] = _build_program(c_main, n_in, n_out, s_guest)
    nc = _BUILD_CACHE[key]

    biasm_host = np.ascontiguousarray(bias.reshape(ob, P).T)  # [P, ob]

    # slot-major assignment: item i -> (core i % 8, slot i // 8)
    plan = [[None] * s_guest for _ in range(n_exp)]
    for i, it in enumerate(items):
        plan[i % N_CORES][i // N_CORES] = it

    in_maps = []
    for e in range(n_exp):
        nm = min(int(g[e]), c_main)
        wt = weight[e].reshape(kb, P, ob, P)
        m = {
            "xm": _tile_x(inputs[offs[e] : offs[e] + nm], n_in, c_main),
            "wk0": np.ascontiguousarray(wt[:, :, :GRP, :]).astype(
                ml_dtypes.bfloat16
            ),
            "wm": np.ascontiguousarray(
                wt[:, :, GRP:, :].transpose(2, 1, 0, 3)
            ).astype(ml_dtypes.bfloat16),
            "biasm": biasm_host,
        }
        if s_guest:
            xg_l, wg_l, bg_l = [], [], []
            for it in plan[e]:
                if it is None:
                    xg_l.append(np.zeros((P, kb, GT), ml_dtypes.bfloat16))
                    wg_l.append(np.zeros((oh, P, kb, P), ml_dtypes.bfloat16))
                    bg_l.append(np.zeros((P, oh), np.float32))
                else:
                    ge, b, h = it
                    t0 = offs[ge] + c_main + b * GT
                    n = min(GT, offs[ge + 1] - t0)
                    xg_l.append(_tile_x(inputs[t0 : t0 + n], n_in, GT))
                    wg_l.append(
                        _tile_w(weight[ge][:, h * oh * P : (h + 1) * oh * P])
                    )
                    bg_l.append(
                        np.ascontiguousarray(
                            bias.reshape(ob, P)[h * oh : (h + 1) * oh].T
                        )
                    )
            m["xg"] = np.stack(xg_l)
            m["wg"] = np.stack(wg_l)
            m["biasg"] = np.stack(bg_l)
        in_maps.append(m)
    meta = (g, offs, plan, c_main, s_guest)
    return nc, in_maps, meta, None, (t_tokens, n_out)


def kernel(inputs, weight, bias, group_sizes):
    nc, in_maps, meta, _, (t_tokens, n_out) = _prepare(
        inputs, weight, bias, group_sizes
    )
    g, offs, plan, c_main, s_guest = meta
    oh = n_out // P // 2
    res = run_bass_kernel_spmd(nc, in_maps, core_ids=list(range(N_CORES)))

    out = np.empty((t_tokens, n_out), np.float32)
    for e in range(N_CORES):
        nm = min(int(g[e]), c_main)
        if nm:
            out[offs[e] : offs[e] + nm] = (
                res.results[e]["ym"][:, :nm].astype(np.float32).T
            )
        for s in range(s_guest):
            it = plan[e][s]
            if it is None:
                continue
            ge, b, h = it
            t0 = offs[ge] + c_main + b * GT
            n = min(GT, int(offs[ge + 1] - t0))
            out[t0 : t0 + n, h * oh * P : (h + 1) * oh * P] = (
                res.results[e]["yg"][s][:, :n].astype(np.float32).T
            )
    return out
